# revision 10
# baseline (speedup 1.0000x reference)
"""Trainium2 Bass kernel for nn_DialogueEmpathyModel.

Strategy (pure data parallelism, B=16 sharded as 2 per core x 8 cores):

- The party GRU + attention never influence the output scores -> dropped.
- The per-speaker global GRU only updates the speaking speaker's row each
  step, so it decomposes into B*SP independent "speaker chains" processed
  in lockstep over occurrence index k (~80 iterations instead of 512x8).
- All input projections (x @ W) are precomputed as batched matmuls
  (phase 0); speaker states are indirect-scattered to time order, then
  batch-projected (phase 1.5); the emotion GRU runs 512 sequential steps
  (phase 2); the output head is batched (phase 3).
- LayerNorm gain is folded into the recurrent weight rows, LayerNorm bias
  is folded through the weights into the precomputed projections; the
  carried state is the pre-affine normalized tensor.
- All matmuls feeding the recurrences use f32r (11-mantissa-bit) hi/lo
  splitting - x@W = xh@Wh + xl@Wh + xh@Wl - restoring ~fp32 accuracy at
  1 cycle/row (native fp32 matmul is 4 cycles/row). This matters: the
  recurrence is chaotic and amplifies any per-step noise >~1e-7 to
  saturation.

The program is JIT-specialized per call on K_PAD (max speaker-chain
length); all data-dependent indexing is host-prepared data (scatter index
tables), so one SPMD program serves all 8 cores.
"""

import numpy as np

H = 512
HB = 2048          # 4H gate width: [r | z | inn | hn]
D = 1024
SP = 8
BSH = 2            # batch per core
NCH = BSH * SP     # speaker chains per core
S = 512
B = 16
NCORES = 8
SR = S * BSH       # time-ordered rows per core
EPS = 1e-5


# ----------------------------------------------------------------------------
# host-side prep
# ----------------------------------------------------------------------------

def _r11(x):
    u = np.ascontiguousarray(x, np.float32).view(np.uint32).astype(np.uint64)
    return (((u + (1 << 11)) >> 12) << 12).astype(np.uint32).view(np.float32)


def _split(x):
    hi = _r11(x)
    lo = _r11(np.asarray(x, np.float32) - hi)
    return hi, lo


def _prep(inputs):
    f32, f64 = np.float32, np.float64
    g = lambda k: np.asarray(inputs[k], f32)
    utt = g("utterances")
    spk = np.asarray(inputs["speaker_ids"]).astype(np.int64)
    gg_wih, gg_whh, gg_bih, gg_bhh = g("gg_wih"), g("gg_whh"), g("gg_bih"), g("gg_bhh")
    eg_wih, eg_whh, eg_bih, eg_bhh = g("eg_wih"), g("eg_whh"), g("eg_bih"), g("eg_bhh")
    ln_g, ln_b = g("ln_g"), g("ln_b")
    out_w1, out_b1 = g("out_w1"), g("out_b1")
    out_ln_g, out_ln_b = g("out_ln_g"), g("out_ln_b")
    out_w2, out_b2 = g("out_w2"), g("out_b2")
    init_global, init_emotion = g("init_global"), g("init_emotion")

    Wx_glob = np.ascontiguousarray(gg_wih[:, :D].T)                     # [D, 3H]
    W_glob = np.concatenate([
        (gg_wih[:H, D:] + gg_whh[:H]).T,
        (gg_wih[H:2 * H, D:] + gg_whh[H:2 * H]).T,
        gg_wih[2 * H:, D:].T,
        gg_whh[2 * H:].T,
    ], axis=1)                                                          # [H, 4H]
    W_globp = (ln_g[:, None].astype(f64) * W_glob.astype(f64)).astype(f32)
    W_emo = np.concatenate([
        (eg_wih[:H, :H] + eg_whh[:H]).T,
        (eg_wih[H:2 * H, :H] + eg_whh[H:2 * H]).T,
        eg_wih[2 * H:, :H].T,
        eg_whh[2 * H:].T,
    ], axis=1)                                                          # [H, 4H]
    W_emop = (ln_g[:, None].astype(f64) * W_emo.astype(f64)).astype(f32)
    W_sx = np.concatenate([
        eg_wih[:H, H:].T,
        eg_wih[H:2 * H, H:].T,
        eg_wih[2 * H:, H:].T,
    ], axis=1)                                                          # [H, 3H]
    W_sxp = (ln_g[:, None].astype(f64) * W_sx.astype(f64)).astype(f32)

    bias_glob = np.concatenate([
        gg_bih[:H] + gg_bhh[:H], gg_bih[H:2 * H] + gg_bhh[H:2 * H],
        gg_bih[2 * H:], gg_bhh[2 * H:]]).astype(f32)                    # [4H]
    bias_emo = np.concatenate([
        eg_bih[:H] + eg_bhh[:H], eg_bih[H:2 * H] + eg_bhh[H:2 * H],
        eg_bih[2 * H:], eg_bhh[2 * H:]]).astype(f32)
    c_b_glob = (ln_b.astype(f64) @ W_glob.astype(f64)).astype(f32)      # [4H]
    c_b_emo = (ln_b.astype(f64) @ W_emo.astype(f64)).astype(f32)
    c_b_sx = np.concatenate([
        (ln_b.astype(f64) @ W_sx.astype(f64)).astype(f32),
        np.zeros(H, f32)])                                              # [4H]

    # chains
    cores = []
    K_max = 0
    for c in range(NCORES):
        bs = [c * BSH + i for i in range(BSH)]
        occ = {}
        for lb, b in enumerate(bs):
            for s in range(SP):
                occ[lb * SP + s] = np.nonzero(spk[b] == s)[0]
        K_max = max(K_max, max(len(v) for v in occ.values()))
        cores.append((bs, occ))
    K_PAD = -(-K_max // 8) * 8
    C0 = K_PAD * NCH

    # shared (core-independent) arrays
    Wx_hi, Wx_lo = _split(Wx_glob)
    Wx2 = np.concatenate([Wx_hi, Wx_lo], axis=0)                        # [2D, 3H]
    c1g = W_globp.astype(f64).sum(0).astype(f32)                        # [4H]
    c1e = W_emop.astype(f64).sum(0).astype(f32)
    Ws_hi, Ws_lo = _split(W_sxp)
    Ws2 = np.concatenate([Ws_hi, Ws_lo], axis=0)                        # [2H, 3H]
    W1T = np.ascontiguousarray(out_w1.T)                                # [H, H2]
    W1_hi, W1_lo = _split(W1T)
    W12 = np.concatenate([W1_hi, W1_lo], axis=0)                        # [2H, H2]
    H2 = W1T.shape[1]

    pxbias = (bias_glob + c_b_glob).astype(f32)
    pxbias_rep = np.broadcast_to(pxbias[:3 * H], (128, 3 * H)).copy()
    biashn_g = np.broadcast_to(pxbias[3 * H:], (NCH, H)).copy()
    sxbias = (bias_emo + c_b_sx + c_b_emo).astype(f32)
    sxbias_rep = np.broadcast_to(sxbias[:3 * H], (128, 3 * H)).copy()
    biashn_e = np.broadcast_to(sxbias[3 * H:], (BSH, H)).copy()
    hb_emo = (init_emotion.astype(f64) @ W_emo.astype(f64)).astype(f32)
    corr0 = np.broadcast_to((hb_emo - c_b_emo), (BSH, HB)).copy()
    h0e = np.broadcast_to(init_emotion, (BSH, H)).copy()
    lng_rep = np.broadcast_to(ln_g, (NCH, H)).copy()
    lnb_rep = np.broadcast_to(ln_b, (NCH, H)).copy()
    b1_rep = np.broadcast_to(out_b1, (128, H2)).copy()
    w2p_rep = np.broadcast_to(out_ln_g * out_w2[0], (128, H2)).copy()
    c2 = float(out_b2[0] + float(out_ln_b.astype(f64) @ out_w2[0].astype(f64)))
    I16 = np.eye(NCH, dtype=f32)
    I2 = np.eye(BSH, dtype=f32)
    I128 = np.eye(128, dtype=f32)

    c1g_rep = np.broadcast_to(c1g, (NCH, HB)).copy()
    c1e_rep = np.broadcast_to(c1e, (BSH, HB)).copy()
    shared = dict(
        Wx2=Wx2, Wg=W_globp, We=W_emop, Ws2=Ws2, W12=W12,
        c1g_rep=c1g_rep, c1e_rep=c1e_rep,
        pxbias_rep=pxbias_rep, biashn_g=biashn_g,
        sxbias_rep=sxbias_rep, biashn_e=biashn_e, corr0=corr0,
        h0e=h0e, lng_rep=lng_rep, lnb_rep=lnb_rep,
        b1_rep=b1_rep, w2p_rep=w2p_rep,
        I16=I16, I2=I2, I128=I128,
    )

    in_maps = []
    for bs, occ in cores:
        xg = np.zeros((K_PAD, NCH, D), f32)
        idx = np.full((NCH, K_PAD), SR, np.int32)   # default -> dump row
        for ch, ts in occ.items():
            lb = ch // SP
            for k, t in enumerate(ts):
                xg[k, ch] = utt[bs[lb], t]
                idx[ch, k] = t * BSH + lb
        xgf = xg.reshape(C0, D)
        xh, xl = _split(xgf)
        xgT2 = np.concatenate(
            [np.ascontiguousarray(xh.T), np.ascontiguousarray(xl.T)], axis=0
        )                                                               # [2D, C0]
        h0g = np.stack([init_global[ch % SP] for ch in range(NCH)])
        px0 = (xg[0].astype(f64) @ Wx_glob.astype(f64)).astype(f32)
        px0 = np.concatenate([px0, np.zeros((NCH, H), f32)], axis=1)
        px0 += bias_glob
        px0 += (h0g.astype(f64) @ W_glob.astype(f64)).astype(f32)
        m = dict(shared)
        m.update(xgT2=xgT2, px0=px0, idx=idx, h0g=h0g)
        in_maps.append({k: np.ascontiguousarray(v) for k, v in m.items()})

    meta = dict(K_PAD=K_PAD, C0=C0, H2=H2, c2=c2)
    return in_maps, meta


# ----------------------------------------------------------------------------
# device program
# ----------------------------------------------------------------------------

def _build(meta):
    import concourse.bass as bass
    import concourse.tile as tile
    from concourse import bacc, mybir

    F32 = mybir.dt.float32
    F32R = mybir.dt.float32r
    I32 = mybir.dt.int32
    AF = mybir.ActivationFunctionType
    OP = mybir.AluOpType

    K_PAD, C0, H2, c2 = meta["K_PAD"], meta["C0"], meta["H2"], meta["c2"]
    NT0 = C0 // 128
    NT = SR // 128

    nc = bacc.Bacc("TRN2", target_bir_lowering=False, debug=False)

    def din(name, shape, dt=F32):
        return nc.dram_tensor(name, shape, dt, kind="ExternalInput")

    xgT2_d = din("xgT2", [2 * D, C0])
    px0_d = din("px0", [NCH, HB])
    idx_d = din("idx", [NCH, K_PAD], I32)
    h0g_d = din("h0g", [NCH, H])
    Wx2_d = din("Wx2", [2 * D, 3 * H])
    Wg_d = din("Wg", [H, HB])
    We_d = din("We", [H, HB])
    c1g_d = din("c1g_rep", [NCH, HB])
    c1e_d = din("c1e_rep", [BSH, HB])
    Ws2_d = din("Ws2", [2 * H, 3 * H])
    W12_d = din("W12", [2 * H, H2])
    pxbias_d = din("pxbias_rep", [128, 3 * H])
    biashn_g_d = din("biashn_g", [NCH, H])
    sxbias_d = din("sxbias_rep", [128, 3 * H])
    biashn_e_d = din("biashn_e", [BSH, H])
    corr0_d = din("corr0", [BSH, HB])
    h0e_d = din("h0e", [BSH, H])
    lng_d = din("lng_rep", [NCH, H])
    lnb_d = din("lnb_rep", [NCH, H])
    b1_d = din("b1_rep", [128, H2])
    w2p_d = din("w2p_rep", [128, H2])
    I16_d = din("I16", [NCH, NCH])
    I2_d = din("I2", [BSH, BSH])
    I128_d = din("I128", [128, 128])

    sc_d = nc.dram_tensor("scores", [SR, 1], F32, kind="ExternalOutput")

    with tile.TileContext(nc) as tc:
        with (
            tc.tile_pool(name="dram", bufs=1, space="DRAM") as dpool,
            tc.tile_pool(name="statics", bufs=1) as spool,
        ):
            px_t = dpool.tile([C0, 3 * H], F32)
            S_t = dpool.tile([SR + 1, H], F32)
            sx_t = dpool.tile([SR, 3 * H], F32)
            eo_t = dpool.tile([SR, H], F32)

            # ---------------- statics ----------------
            I16_sb = spool.tile([NCH, NCH], F32)
            nc.sync.dma_start(I16_sb[:], I16_d[:])
            I2_sb = spool.tile([BSH, BSH], F32)
            nc.sync.dma_start(I2_sb[:], I2_d[:])
            I128_sb = spool.tile([128, 128], F32)
            nc.sync.dma_start(I128_sb[:], I128_d[:])
            idx_sb = spool.tile([NCH, K_PAD], I32)
            nc.sync.dma_start(idx_sb[:], idx_d[:])
            lng_sb = spool.tile([NCH, H], F32)
            nc.sync.dma_start(lng_sb[:], lng_d[:])
            lnb_sb = spool.tile([NCH, H], F32)
            nc.sync.dma_start(lnb_sb[:], lnb_d[:])
            biashn_g_sb = spool.tile([NCH, H], F32)
            nc.sync.dma_start(biashn_g_sb[:], biashn_g_d[:])
            biashn_e_sb = spool.tile([BSH, H], F32)
            nc.sync.dma_start(biashn_e_sb[:], biashn_e_d[:])
            eps_sb = spool.tile([128, 1], F32)
            nc.vector.memset(eps_sb[:], EPS)
            c2_sb = spool.tile([128, 1], F32)
            nc.vector.memset(c2_sb[:], c2)

            # ================ phase 0: px = xg @ Wx (split) ================
            with (
                nc.named_scope("phase0"),
                tc.tile_pool(name="p0w", bufs=1) as p0w,
                tc.tile_pool(name="p0", bufs=3) as p0,
                tc.tile_pool(name="p0ps", bufs=2, space="PSUM") as p0ps,
            ):
                Wx_sb = p0w.tile([128, 16 * 3 * H], F32R)
                for c in range(16):
                    nc.sync.dma_start(
                        Wx_sb[:, c * 3 * H:(c + 1) * 3 * H],
                        Wx2_d[128 * c:128 * (c + 1), :].bitcast(F32R),
                    )
                pxbias_sb = p0w.tile([128, 3 * H], F32)
                nc.sync.dma_start(pxbias_sb[:], pxbias_d[:])

                for mt in range(NT0):
                    xg_sb = p0.tile([128, 16 * 128], F32R, tag="xg")
                    for c in range(16):
                        nc.sync.dma_start(
                            xg_sb[:, 128 * c:128 * (c + 1)],
                            xgT2_d[
                                128 * c:128 * (c + 1), 128 * mt:128 * (mt + 1)
                            ].bitcast(F32R),
                        )
                    ps = p0ps.tile([128, 3 * H], F32, tag="ps")
                    # (src_chunk in xg_sb, w_chunk in Wx_sb) per term
                    pairs = (
                        [(j, j) for j in range(8)]            # xh @ Wh
                        + [(8 + j, j) for j in range(8)]      # xl @ Wh
                        + [(j, 8 + j) for j in range(8)]      # xh @ Wl
                    )
                    for blk in range(3):
                        for i, (sc_, wc) in enumerate(pairs):
                            nc.tensor.matmul(
                                ps[:, 512 * blk:512 * (blk + 1)],
                                xg_sb[:, 128 * sc_:128 * (sc_ + 1)],
                                Wx_sb[:, 3 * H * wc + 512 * blk:3 * H * wc + 512 * (blk + 1)],
                                start=(i == 0),
                                stop=(i == len(pairs) - 1),
                            )
                    px_out = p0.tile([128, 3 * H], F32, tag="pxo")
                    nc.vector.tensor_add(px_out[:], ps[:], pxbias_sb[:])
                    nc.sync.dma_start(px_t[128 * mt:128 * (mt + 1), :], px_out[:])

            # ============ phase 1: speaker chains (K_PAD iters) ============
            # phase 1 wiring
            px0_sb = spool.tile([NCH, HB], F32)
            nc.sync.dma_start(px0_sb[:], px0_d[:])

            def scatter_g(k, xc):
                nc.gpsimd.indirect_dma_start(
                    out=S_t[:],
                    out_offset=bass.IndirectOffsetOnAxis(
                        ap=idx_sb[:, k:k + 1], axis=0
                    ),
                    in_=xc[:],
                    in_offset=None,
                )

            class P1Src:
                def __call__(self, k):
                    if k == 0:
                        return px0_d[:, 0:3 * H], None
                    return px_t[NCH * k:NCH * (k + 1), :], None

            def hn_src_sel(k):
                return px0_sb[:, 3 * H:HB] if k == 0 else biashn_g_sb[:]

            def gru_v2(n_rows, n_iters, W_d, c1_d, Ipart_sb, px_src,
                       hn_src_f, h0_src, scatter, eo_store, lng2, lnb2):
                with (
                    tc.tile_pool(name="w", bufs=1) as wpool,
                    tc.tile_pool(name="st", bufs=2) as st,
                    tc.tile_pool(name="wk", bufs=2) as wk,
                    tc.tile_pool(name="pxp", bufs=4) as pxp,
                    tc.tile_pool(name="psg", bufs=1, space="PSUM") as psg,
                    tc.tile_pool(name="pst", bufs=2, space="PSUM") as pst,
                ):
                    W_sb = wpool.tile([128, 4 * HB], F32)
                    for c in range(4):
                        nc.sync.dma_start(
                            W_sb[:, HB * c:HB * (c + 1)],
                            W_d[128 * c:128 * (c + 1), :],
                        )
                    c1_sb = wpool.tile([n_rows, HB], F32)
                    nc.sync.dma_start(c1_sb[:], c1_d[:])
                    h_full = st.tile([n_rows, H], F32, tag="hf")
                    nc.sync.dma_start(h_full[:], h0_src)
                    hrT = st.tile([128, 4 * n_rows], F32, tag="hrT")
                    nc.vector.memset(hrT[:], 0.0)
                    rstd = st.tile([n_rows, 1], F32, tag="rstd")
                    nc.vector.memset(rstd[:], 1.0)
                    negmr = st.tile([n_rows, 1], F32, tag="negmr")
                    nc.vector.memset(negmr[:], 0.0)

                    for k in range(n_iters):
                        src_ap, extra = px_src(k)
                        px_sb = pxp.tile([n_rows, 3 * H], F32, tag="px")
                        nc.sync.dma_start(px_sb[:], src_ap)
                        # B = px' + (-mu*rstd)_prev * colsum(W') : the LN mean
                        # correction folded into the projection constants
                        B = wk.tile([n_rows, HB], F32, tag="B")
                        nc.vector.scalar_tensor_tensor(
                            B[:, 0:3 * H], c1_sb[:, 0:3 * H], negmr[:],
                            px_sb[:], OP.mult, OP.add,
                        )
                        nc.vector.scalar_tensor_tensor(
                            B[:, 3 * H:HB], c1_sb[:, 3 * H:HB], negmr[:],
                            hn_src_f(k), OP.mult, OP.add,
                        )
                        if extra is not None:
                            nc.vector.tensor_add(B[:], B[:], extra)

                        # h_raw^T @ W' : fp32, 4 col-groups round-robin,
                        # one PSUM bank per group (gate)
                        ps4 = psg.tile([128, HB], F32, tag="ps4")
                        for kc in range(4):
                            for gq in range(4):
                                nc.tensor.matmul(
                                    ps4[32 * gq:32 * gq + n_rows,
                                        512 * gq:512 * (gq + 1)],
                                    hrT[:, n_rows * kc:n_rows * (kc + 1)],
                                    W_d_slice(W_sb, kc, gq),
                                    start=(kc == 0),
                                    stop=(kc == 3),
                                    tile_position=(0, 32 * gq),
                                )
                        # gates: gpre = rstd_prev * psum + B
                        gr = wk.tile([n_rows, H], F32, tag="gr")
                        nc.vector.scalar_tensor_tensor(
                            gr[:], ps4[0:n_rows, 0:512], rstd[:],
                            B[:, 0:512], OP.mult, OP.add,
                        )
                        r_sg = wk.tile([n_rows, H], F32, tag="r")
                        nc.scalar.activation(r_sg[:], gr[:], AF.Sigmoid)
                        ghn = wk.tile([n_rows, H], F32, tag="ghn")
                        nc.vector.scalar_tensor_tensor(
                            ghn[:], ps4[96:96 + n_rows, 1536:2048], rstd[:],
                            B[:, 1536:2048], OP.mult, OP.add,
                        )
                        gz = wk.tile([n_rows, H], F32, tag="gz")
                        nc.vector.scalar_tensor_tensor(
                            gz[:], ps4[32:32 + n_rows, 512:1024], rstd[:],
                            B[:, 512:1024], OP.mult, OP.add,
                        )
                        z_sg = wk.tile([n_rows, H], F32, tag="z")
                        nc.scalar.activation(z_sg[:], gz[:], AF.Sigmoid)
                        zhf = wk.tile([n_rows, H], F32, tag="zhf")
                        nc.vector.tensor_mul(zhf[:], z_sg[:], h_full[:])
                        rhn = wk.tile([n_rows, H], F32, tag="rhn")
                        nc.vector.tensor_mul(rhn[:], r_sg[:], ghn[:])
                        ginn = wk.tile([n_rows, H], F32, tag="ginn")
                        nc.vector.scalar_tensor_tensor(
                            ginn[:], ps4[64:64 + n_rows, 1024:1536], rstd[:],
                            B[:, 1024:1536], OP.mult, OP.add,
                        )
                        npre = wk.tile([n_rows, H], F32, tag="npre")
                        nc.vector.tensor_add(npre[:], rhn[:], ginn[:])
                        n_t = wk.tile([n_rows, H], F32, tag="n")
                        nc.scalar.activation(n_t[:], npre[:], AF.Tanh)
                        zn = wk.tile([n_rows, H], F32, tag="zn")
                        nc.vector.tensor_mul(zn[:], z_sg[:], n_t[:])
                        u_t = wk.tile([n_rows, H], F32, tag="u")
                        nc.vector.tensor_sub(u_t[:], n_t[:], zn[:])
                        h_raw = wk.tile([n_rows, H], F32, tag="hraw")
                        nc.vector.tensor_add(h_raw[:], u_t[:], zhf[:])
                        if eo_store is not None:
                            nc.sync.dma_start(eo_store(k), h_raw[:])
                        # transpose h_raw for the next matmul (pre-LN; the
                        # scale/mean corrections apply post-matmul next iter)
                        pt = pst.tile([128, 4 * n_rows], F32, tag="pt")
                        for j in range(4):
                            nc.tensor.transpose(
                                pt[:, n_rows * j:n_rows * (j + 1)],
                                h_raw[:, 128 * j:128 * (j + 1)],
                                Ipart_sb[:],
                            )
                        hrT = st.tile([128, 4 * n_rows], F32, tag="hrT")
                        nc.vector.tensor_copy(hrT[:], pt[:])
                        # LN stats (off the critical path)
                        st6 = wk.tile([n_rows, 6], F32, tag="st6")
                        nc.vector.bn_stats(st6[:], h_raw[:])
                        st2 = wk.tile([n_rows, 2], F32, tag="st2")
                        nc.vector.bn_aggr(st2[:], st6[:])
                        sd = wk.tile([n_rows, 1], F32, tag="sd")
                        nc.scalar.activation(sd[:], st2[:, 1:2], AF.Sqrt,
                                             bias=eps_sb[0:n_rows, :])
                        rstd = st.tile([n_rows, 1], F32, tag="rstd")
                        nc.vector.reciprocal(rstd[:], sd[:])
                        negmr = st.tile([n_rows, 1], F32, tag="negmr")
                        nc.vector.tensor_scalar(
                            negmr[:], st2[:, 0:1], rstd[:], -1.0,
                            OP.mult, OP.mult,
                        )
                        xc = wk.tile([n_rows, H], F32, tag="xc")
                        nc.vector.tensor_scalar(
                            xc[:], h_raw[:], st2[:, 0:1], rstd[:],
                            OP.subtract, OP.mult,
                        )
                        h_full = st.tile([n_rows, H], F32, tag="hf")
                        nc.vector.tensor_mul(h_full[:], xc[:], lng2)
                        nc.vector.tensor_add(h_full[:], h_full[:], lnb2)
                        if scatter is not None:
                            scatter(k, xc)

            def W_d_slice(W_sb, kc, gq):
                return W_sb[:, HB * kc + 512 * gq:HB * kc + 512 * (gq + 1)]

            with nc.named_scope("phase1"):
                gru_v2(
                    NCH, K_PAD, Wg_d, c1g_d, I16_sb, P1Src(), hn_src_sel,
                    h0g_d[:], scatter_g, None, lng_sb[:], lnb_sb[:],
                )

            # ============ phase 1.5: sx = S @ Wsx (split) + bias ============
            with (
                nc.named_scope("phase15"),
                tc.tile_pool(name="p15w", bufs=1) as p15w,
                tc.tile_pool(name="p15", bufs=3) as p15,
                tc.tile_pool(name="p15pt", bufs=2, space="PSUM") as p15pt,
                tc.tile_pool(name="p15ps", bufs=2, space="PSUM") as p15ps,
            ):
                Ws_sb = p15w.tile([128, 8 * 3 * H], F32R)
                for c in range(8):
                    nc.sync.dma_start(
                        Ws_sb[:, 3 * H * c:3 * H * (c + 1)],
                        Ws2_d[128 * c:128 * (c + 1), :].bitcast(F32R),
                    )
                sxbias_sb = p15w.tile([128, 3 * H], F32)
                nc.sync.dma_start(sxbias_sb[:], sxbias_d[:])

                for mt in range(NT):
                    s_sb = p15.tile([128, H], F32, tag="s")
                    nc.sync.dma_start(s_sb[:], S_t[128 * mt:128 * (mt + 1), :])
                    ptr = p15pt.tile([128, H], F32, tag="ptr")
                    for j in range(4):
                        nc.tensor.transpose(
                            ptr[:, 128 * j:128 * (j + 1)],
                            s_sb[:, 128 * j:128 * (j + 1)],
                            I128_sb[:],
                        )
                    sTh = p15.tile([128, H], F32R, tag="sTh")
                    nc.vector.tensor_copy(sTh[:], ptr[:])
                    sTl = p15.tile([128, H], F32R, tag="sTl")
                    nc.vector.tensor_sub(sTl[:], ptr[:], sTh[:])
                    ps = p15ps.tile([128, 3 * H], F32, tag="ps")
                    kcs = (
                        [(sTh, j, j) for j in range(4)]
                        + [(sTl, j, j) for j in range(4)]
                        + [(sTh, j, 4 + j) for j in range(4)]
                    )
                    for blk in range(3):
                        for i, (src_t, jc, wc) in enumerate(kcs):
                            nc.tensor.matmul(
                                ps[:, 512 * blk:512 * (blk + 1)],
                                src_t[:, 128 * jc:128 * (jc + 1)],
                                Ws_sb[:, 3 * H * wc + 512 * blk:3 * H * wc + 512 * (blk + 1)],
                                start=(i == 0),
                                stop=(i == len(kcs) - 1),
                            )
                    sx_out = p15.tile([128, 3 * H], F32, tag="sxo")
                    nc.vector.tensor_add(sx_out[:], ps[:], sxbias_sb[:])
                    nc.sync.dma_start(sx_t[128 * mt:128 * (mt + 1), :], sx_out[:])

            # ================ phase 2: emo chain (S iters) ================
            corr0_sb = spool.tile([BSH, HB], F32)
            nc.sync.dma_start(corr0_sb[:], corr0_d[:])

            def px_src_e(t):
                return sx_t[BSH * t:BSH * (t + 1), :], (
                    corr0_sb[:, :] if t == 0 else None
                )

            def eo_store_e(t):
                return eo_t[BSH * t:BSH * (t + 1), :]

            with nc.named_scope("phase2"):
                gru_v2(
                    BSH, S, We_d, c1e_d, I2_sb, px_src_e,
                    lambda t: biashn_e_sb[:],
                    h0e_d[:], None, eo_store_e,
                    lng_sb[0:BSH, :], lnb_sb[0:BSH, :],
                )

            # ================ phase 3: output head ================
            with (
                nc.named_scope("phase3"),
                tc.tile_pool(name="p3w", bufs=1) as p3w,
                tc.tile_pool(name="p3", bufs=3) as p3,
                tc.tile_pool(name="p3pt", bufs=2, space="PSUM") as p3pt,
                tc.tile_pool(name="p3ps", bufs=2, space="PSUM") as p3ps,
            ):
                W1_sb = p3w.tile([128, 8 * H2], F32R)
                for c in range(8):
                    nc.sync.dma_start(
                        W1_sb[:, H2 * c:H2 * (c + 1)],
                        W12_d[128 * c:128 * (c + 1), :].bitcast(F32R),
                    )
                b1_sb = p3w.tile([128, H2], F32)
                nc.sync.dma_start(b1_sb[:], b1_d[:])
                w2p_sb = p3w.tile([128, H2], F32)
                nc.sync.dma_start(w2p_sb[:], w2p_d[:])

                for mt in range(NT):
                    e_sb = p3.tile([128, H], F32, tag="e")
                    nc.sync.dma_start(e_sb[:], eo_t[128 * mt:128 * (mt + 1), :])
                    ptr = p3pt.tile([128, H], F32, tag="ptr")
                    for j in range(4):
                        nc.tensor.transpose(
                            ptr[:, 128 * j:128 * (j + 1)],
                            e_sb[:, 128 * j:128 * (j + 1)],
                            I128_sb[:],
                        )
                    eTh = p3.tile([128, H], F32R, tag="eTh")
                    nc.vector.tensor_copy(eTh[:], ptr[:])
                    eTl = p3.tile([128, H], F32R, tag="eTl")
                    nc.vector.tensor_sub(eTl[:], ptr[:], eTh[:])
                    ps1 = p3ps.tile([128, H2], F32, tag="ps1")
                    kcs = (
                        [(eTh, j, j) for j in range(4)]
                        + [(eTl, j, j) for j in range(4)]
                        + [(eTh, j, 4 + j) for j in range(4)]
                    )
                    for i, (src_t, jc, wc) in enumerate(kcs):
                        nc.tensor.matmul(
                            ps1[:, :],
                            src_t[:, 128 * jc:128 * (jc + 1)],
                            W1_sb[:, H2 * wc:H2 * (wc + 1)],
                            start=(i == 0),
                            stop=(i == len(kcs) - 1),
                        )
                    nc.vector.tensor_add(ps1[:], ps1[:], b1_sb[:])
                    h1 = p3.tile([128, H2], F32, tag="h1")
                    nc.scalar.activation(h1[:], ps1[:], AF.Relu)
                    st6 = p3.tile([128, 6], F32, tag="st6")
                    nc.vector.bn_stats(st6[:], h1[:])
                    st2 = p3.tile([128, 2], F32, tag="st2")
                    nc.vector.bn_aggr(st2[:], st6[:])
                    sd = p3.tile([128, 1], F32, tag="sd")
                    nc.scalar.activation(sd[:], st2[:, 1:2], AF.Sqrt, bias=eps_sb[:])
                    rstd = p3.tile([128, 1], F32, tag="rstd")
                    nc.vector.reciprocal(rstd[:], sd[:])
                    xc1 = p3.tile([128, H2], F32, tag="xc1")
                    nc.vector.tensor_scalar(
                        xc1[:], h1[:], st2[:, 0:1], rstd[:],
                        OP.subtract, OP.mult,
                    )
                    scr = p3.tile([128, H2], F32, tag="scr")
                    spre = p3.tile([128, 1], F32, tag="spre")
                    nc.vector.scalar_tensor_tensor(
                        scr[:], xc1[:], 0.0, w2p_sb[:],
                        OP.bypass, OP.mult, accum_out=spre[:],
                    )
                    sco = p3.tile([128, 1], F32, tag="sco")
                    nc.scalar.activation(sco[:], spre[:], AF.Sigmoid, bias=c2_sb[:])
                    nc.sync.dma_start(sc_d[128 * mt:128 * (mt + 1), :], sco[:])

    nc.compile()
    return nc


# ----------------------------------------------------------------------------
# entry point
# ----------------------------------------------------------------------------

_cache = {}


def kernel(**inputs):
    from concourse.bass_utils import run_bass_kernel_spmd

    in_maps, meta = _prep(inputs)
    key = (meta["K_PAD"], meta["H2"], round(meta["c2"], 10))
    if key not in _cache:
        _cache[key] = _build(meta)
    nc = _cache[key]

    res = run_bass_kernel_spmd(nc, in_maps, list(range(NCORES)), trace=False)
    out = np.zeros((B, S, 1), np.float32)
    for c in range(NCORES):
        sc = res.results[c]["scores"].reshape(SR)
        for lb in range(BSH):
            out[c * BSH + lb, :, 0] = sc[lb::BSH]
    return out

